# revision 18
# baseline (speedup 1.0000x reference)
"""Trainium2 Bass kernel for nn_ELM_AE_FatSpectral_Ensemble.

Data-parallel over batch: 4 samples/core on 8 cores.  fp16 data path:
  x is cast fp32->fp16 during the input DMA (SWDGE), transposed to x^T via
  DMA XBAR transpose, resized on PE (XrT = Rk^T x^T).  zscore is folded:
    H   = sigmoid(isd .* resize(Wc @ x))     (Wc = W - rowmean(W), host-prep)
    C^T = (isd.*Hblk)^T XrT - corr x (-1 row)   (corr row at partition 96)
  Per super (2 samples): G = H H^T accumulated in PSUM as a 128x128
  block-diagonal supermatrix; Newton-Schulz inverse in fp32, interleaved at
  emission with the next super's processing.  Members are processed in two
  passes per super (A: loads/resize/stats+sqrt, B: sigmoid/G/C^T) so the
  scalar engine never alternates activation tables within a pass.
  m3 (c=2048, 7x7) is loaded packed ((p k) channel order) so its DMA uses
  1568-B lines instead of 98-B; its channel permutation is undone by an
  on-chip gather before the output DMA.
"""

import numpy as np

import concourse.bacc as bacc
import concourse.tile as tile
from concourse import mybir
from concourse.bass_utils import run_bass_kernel_spmd

F32 = mybir.dt.float32
F32R = mybir.dt.float32r
F16 = mybir.dt.float16
AF = mybir.ActivationFunctionType
ALU = mybir.AluOpType
NPF16 = np.float16

S = 4
NCORES = 8
SP = 14
WH = SP * SP
Q = 16
MEMBERS = [(256, 56), (512, 28), (1024, 14), (2048, 7)]
OFFS = [0, 256, 768, 1792]
DTOT = 3840
ORDER = [3, 2, 1, 0]
NEWTON_ITERS = 10


def _weight_mat(n_in, n_out):
    scale = n_out / n_in
    kernel_scale = max(1.0, 1.0 / scale)
    sample_f = (np.arange(n_out) + 0.5) / scale - 0.5
    x = np.abs(sample_f[:, None] - np.arange(n_in)[None, :]) / kernel_scale
    w = np.maximum(0.0, 1.0 - x)
    total = w.sum(axis=1, keepdims=True)
    return (w / np.where(total > 0, total, 1)).astype(np.float32)


def _chunks(n, sz=128):
    return [(i, min(sz, n - i)) for i in range(0, n, sz)]


def _consts():
    rts = {}
    for m, (c, sp) in enumerate(MEMBERS):
        if sp == SP:
            continue
        R = _weight_mat(sp, SP)
        rts[m] = np.kron(R, R).T.astype(np.float32)   # [uv, 196]
    rtb = {}
    for m in (0, 1):
        rt = rts[m]
        pad = (-rt.shape[0]) % 128
        rtp = np.concatenate([rt, np.zeros((pad, WH), np.float32)], 0)
        rtb[m] = np.ascontiguousarray(
            rtp.reshape(-1, 128, WH).transpose(1, 0, 2).astype(NPF16))
    # m3: rows 0:49 and 64:113 both hold rt3 (for base-64 parity matmuls)
    rt3 = np.zeros((128, 1, WH), np.float32)
    rt3[0:49, 0] = rts[3]
    rt3[64:113, 0] = rts[3]
    rtb[3] = np.ascontiguousarray(rt3.astype(NPF16))
    ident = np.eye(128, dtype=np.float32)
    identh = np.eye(128, dtype=NPF16)
    p16 = np.kron(np.eye(8, dtype=np.float32), np.ones((16, 16), np.float32))
    mask8 = np.zeros((4, 2, 128, 2), np.float32)
    for m in range(4):
        for spr in range(2):
            for si in range(2):
                b = 4 * si + m
                mask8[m, spr, 16 * b:16 * b + 16, si] = 1.0
    mask8 = np.ascontiguousarray(mask8.transpose(2, 0, 1, 3))  # [128,4,2,2]
    return rts, rtb, ident, identh, p16, mask8


def _drive(gen, n):
    if gen is None:
        return False
    for _ in range(n):
        try:
            next(gen)
        except StopIteration:
            return False
    return True


def _build_program():
    rts, rtb, ident_np, identh_np, p16_np, mask8_np = _consts()

    nc = bacc.Bacc()
    xin, waugd, rtd = {}, {}, {}
    for m, (c, sp) in enumerate(MEMBERS):
        uv = sp * sp
        xin[m] = nc.dram_tensor(f"x{m}", [S, c, uv], F32R, kind="ExternalInput")
        nw = 16 if m == 3 else c // 128
        waugd[m] = nc.dram_tensor(f"waug{m}", [128, nw, Q], F16,
                                  kind="ExternalInput")
        if m in rtb:
            rtd[m] = nc.dram_tensor(f"rt{m}", list(rtb[m].shape), F16,
                                    kind="ExternalInput")
    identd = nc.dram_tensor("ident", [128, 128], F32, kind="ExternalInput")
    identhd = nc.dram_tensor("identh", [128, 128], F16, kind="ExternalInput")
    p16d = nc.dram_tensor("p16", [128, 128], F32, kind="ExternalInput")
    mask8d = nc.dram_tensor("mask8", [128, 4, 2, 2], F32, kind="ExternalInput")
    outd = nc.dram_tensor("out", [S, DTOT], F32, kind="ExternalOutput")

    nzk = {}
    for m in (0, 1):
        uv = MEMBERS[m][1] ** 2
        nzk[m] = {}
        for Mi, (Mo, Msz) in enumerate([(0, 128), (128, 68)]):
            nzk[m][Mi] = [ki for ki, (ko, ksz) in enumerate(_chunks(uv))
                          if np.any(rts[m][ko:ko + ksz, Mo:Mo + Msz] != 0)]

    from contextlib import ExitStack
    _ceng = [0]
    _teng = [0]

    def _pcopy(out, in_):
        _ceng[0] ^= 1
        if _ceng[0]:
            nc.scalar.copy(out=out, in_=in_)
        else:
            nc.vector.tensor_copy(out=out, in_=in_)

    def _tdma(out, in_):
        nc.sync.dma_start(out=out, in_=in_, transpose=True)

    with tile.TileContext(nc) as tc, ExitStack() as _es:
        _p = lambda **kw: _es.enter_context(tc.tile_pool(**kw))
        consts = _p(name="consts", bufs=1)
        xbfp = _p(name="xbfp", bufs=1)
        xtp = _p(name="xtp", bufs=1)
        xrtp = _p(name="xrtp", bufs=1)
        pstp = _p(name="pstp", bufs=1)
        pctp = _p(name="pctp", bufs=1)
        prcp = _p(name="prcp", bufs=1)
        hp = _p(name="hp", bufs=2)
        smalls = _p(name="smalls", bufs=3)
        sup = _p(name="sup", bufs=1)
        newt = _p(name="newt", bufs=2)
        outp = _p(name="outp", bufs=2)
        pacc = _p(name="pacc", bufs=2, space="PSUM")   # [128,1024]: 4 banks
        ps3 = _p(name="ps3", bufs=3, space="PSUM")     # 3 banks shared tag
        pg = _p(name="pg", bufs=1, space="PSUM")       # 1 bank

        # ---------------- constants ----------------
        ident_sb = consts.tile([128, 128], F32, tag="ident")
        nc.sync.dma_start(out=ident_sb, in_=identd[:, :])
        identh_sb = consts.tile([128, 128], F16, tag="identh")
        nc.sync.dma_start(out=identh_sb, in_=identhd[:, :])
        p16_sb = consts.tile([128, 128], F32, tag="p16")
        nc.sync.dma_start(out=p16_sb, in_=p16d[:, :])
        mask8_sb = consts.tile([128, 4, 2, 2], F32, tag="mask8")
        nc.sync.dma_start(out=mask8_sb, in_=mask8d[:, :, :, :])
        oq8_sb = consts.tile([128, 4, 2, 2], F16, tag="oq8")
        nc.vector.tensor_scalar_mul(
            oq8_sb.rearrange("p a b c -> p (a b c)"),
            mask8_sb.rearrange("p a b c -> p (a b c)"), 1.0 / (Q - 1))
        ones_sb = consts.tile([128, 1], F32, tag="ones")
        nc.vector.memset(ones_sb, 1.0)

        rtt, waug_sb = {}, {}
        for m, (c, sp) in enumerate(MEMBERS):
            nw = 16 if m == 3 else c // 128
            waug_sb[m] = consts.tile([128, nw, Q], F16, tag=f"waug{m}",
                                     name=f"waug{m}")
            nc.sync.dma_start(out=waug_sb[m], in_=waugd[m][:, :, :])
            if m in rtb:
                nk = rtb[m].shape[1]
                rtt[m] = consts.tile([128, nk, WH], F16, tag=f"rt{m}",
                                     name=f"rt{m}")
                nc.sync.dma_start(out=rtt[m], in_=rtd[m][:, :, :])

        ct_all = []
        for spr in range(2):
            t = sup.tile([128, DTOT], F16, tag=f"ct{spr}", name=f"ct{spr}")
            ct_all.append(t)
        g_sb = [None, None]
        m2_sb = [None, None]
        r_sb = [None, None]
        r4_all = [[None] * 4, [None] * 4]

        # ---------- pass A: load/transpose/P/resize/stats ----------
        def pass_a(spr, m, xbf, st_out):
            c, sp = MEMBERS[m]
            uv = sp * sp
            ccn = 16 if m == 3 else c // 128
            if m != 3:
                kch = _chunks(uv)
                nk = len(kch)

            # x^T via DMA XBAR transpose
            xts = {}
            for si in range(2):
                if m == 3:
                    xt = xtp.tile([128, 8, 128], F16, tag=f"xt3_{si}",
                                  name=f"xt3_{si}")
                    _tdma(out=xt, in_=xbf[si].rearrange("p a b -> p (a b)"))
                    xb = xtp.tile([49, 8, 128], F16, tag=f"xt3b_{si}",
                                  name=f"xt3b_{si}")
                    nc.vector.tensor_copy(out=xb, in_=xt[64:113, :, :])
                    xts[si, "b"] = xb
                else:
                    xt = xtp.tile([128, ccn, nk, 128], F16, tag=f"xt{m}_{si}",
                                  name=f"xt{m}_{si}")
                    for cc in range(ccn):
                        _tdma(out=xt[:, cc, :, :], in_=xbf[si][:, cc, :])
                xts[si] = xt

            # P = Wc @ x
            uvp = 128 if m == 3 else nk * 128
            pst = {}
            for si in range(2):
                pst[si] = pstp.tile([16, uvp], F16, tag=f"pst{m}_{si}",
                                    name=f"pst{m}_{si}")
                if uv < uvp:
                    nc.vector.memset(pst[si][:, uv:uvp], 0.0)
                for no, nsz in _chunks(uv, 512):
                    pp_ps = ps3.tile([16, 512], F32, tag="ps")
                    for cc in range(ccn):
                        nc.tensor.matmul(
                            pp_ps[:, :nsz], lhsT=waug_sb[m][:, cc, :],
                            rhs=xbf[si][:, cc, no:no + nsz],
                            start=(cc == 0), stop=(cc == ccn - 1))
                    _pcopy(out=pst[si][:, no:no + nsz],
                           in_=pp_ps[:Q, :nsz])

            # Pc^T (PE transposes, small)
            pnk = 1 if m == 3 else nk
            pct_t = pctp.tile([128, pnk, 32], F16, tag=f"pct{m}",
                              name=f"pct{m}")
            for ki in range(pnk):
                tp_ps = ps3.tile([128, 32], F16, tag="ps")
                for si in range(2):
                    nc.tensor.transpose(
                        tp_ps[:, 16 * si:16 * si + 16],
                        pst[si][:, ki * 128:(ki + 1) * 128],
                        identh_sb[:16, :16])
                _pcopy(out=pct_t[:, ki, :], in_=tp_ps)

            # resize -> XrT (m2: direct view of x^T)
            xrt = {}
            if m == 2:
                for si in range(2):
                    nc.vector.memset(xts[si][96:97, :, 1, :], -1.0)
            else:
                for si in range(2):
                    x0t = xrtp.tile([128, c], F16, tag=f"x0t{m}_{si}",
                                    name=f"x0t{m}_{si}")
                    x1t = xrtp.tile([97, c], F16, tag=f"x1t{m}_{si}",
                                    name=f"x1t{m}_{si}")
                    nc.vector.memset(x1t[64:97, :], 0.0)
                    nc.vector.memset(x1t[96:97, :], -1.0)
                    for wi, (Mo, Msz, dst) in enumerate(
                            ((0, 128, x0t), (128, 68, x1t))):
                        if m == 3:
                            for e in range(2):
                                src3 = (xts[si] if e == 0
                                        else xts[si, "b"])
                                pr = pacc.tile([128, 1024], F32, tag="pacc")
                                for h in range(2):
                                    nc.tensor.matmul(
                                        pr[:Msz, 512 * h:512 * (h + 1)],
                                        lhsT=rtt[3][0:49, 0, Mo:Mo + Msz],
                                        rhs=src3[0:49, 4 * h:4 * h + 4, :],
                                        start=True, stop=True)
                                _pcopy(out=dst[:Msz,
                                               1024 * e:1024 * (e + 1)],
                                       in_=pr[:Msz, :])
                        else:
                            for no, nsz in _chunks(c, 1024):
                                pr = pacc.tile([128, 1024], F32, tag="pacc")
                                ks = nzk[m][wi]
                                for so, ssz in _chunks(nsz, 512):
                                    cc0 = (no + so) // 128
                                    ncc = (ssz + 127) // 128
                                    for idx, ki in enumerate(ks):
                                        ko, ksz = kch[ki]
                                        nc.tensor.matmul(
                                            pr[:Msz, so:so + ssz],
                                            lhsT=rtt[m][:ksz, ki,
                                                        Mo:Mo + Msz],
                                            rhs=xts[si][:ksz, cc0:cc0 + ncc,
                                                        ki, :],
                                            start=(idx == 0),
                                            stop=(idx == len(ks) - 1))
                                _pcopy(out=dst[:Msz, no:no + nsz],
                                       in_=pr[:Msz, :nsz])
                    xrt[si, 0] = x0t
                    xrt[si, 1] = x1t

            def _xview(si, wi, no, nsz):
                hi = 128 if wi == 0 else 97
                if m != 2:
                    return xrt[si, wi][:hi, no:no + nsz]
                cc0 = no // 128
                ncc = (nsz + 127) // 128
                return xts[si][:hi, cc0:cc0 + ncc, wi, :]

            # stats: mu, isd, v = isd*mu
            stats = {}
            for si in range(2):
                for wi, psz in ((0, 128), (1, 68)):
                    if m != 2:
                        t = xrt[si, wi]
                        srcs = [t[:psz, go:go + gln]
                                for go, gln in _chunks(c, 512)]
                    else:
                        srcs = [xts[si][:psz, cc, wi, :]
                                for cc in range(c // 128)]
                    st = smalls.tile([128, len(srcs), 6], F32, tag="bnst")
                    for gi, src in enumerate(srcs):
                        nc.vector.bn_stats(out=st[:psz, gi, :], in_=src)
                    mv = smalls.tile([128, 2], F32, tag="mv")
                    nc.vector.bn_aggr(out=mv[:psz], in_=st[:psz])
                    sd = smalls.tile([128, 1], F32, tag="sd")
                    nc.scalar.activation(out=sd[:psz], in_=mv[:psz, 1:2],
                                         func=AF.Sqrt, scale=c / (c - 1.0))
                    isd = smalls.tile([128, 1], F32, tag="isd", bufs=18)
                    nc.vector.reciprocal(out=isd[:psz], in_=sd[:psz])
                    v = smalls.tile([128, 1], F16, tag="vv", bufs=18)
                    nc.vector.tensor_mul(v[:psz], mv[:psz, 0:1], isd[:psz])
                    stats[si, wi] = (isd, v)

            # PRc = Rk^T Pc^T -> SBUF fp32 (m2: pct_t used directly)
            prc_sb = None
            if m != 2:
                prc_sb = prcp.tile([128, 2, 32], F32, tag=f"prc{m}",
                                   name=f"prc{m}")
                for wi, (Mo, Msz) in enumerate(((0, 128), (128, 68))):
                    ps = ps3.tile([128, 32], F32, tag="ps")
                    if m == 3:
                        nc.tensor.matmul(
                            ps[:Msz, :], lhsT=rtt[3][0:49, 0, Mo:Mo + Msz],
                            rhs=pct_t[0:49, 0, :], start=True, stop=True)
                    else:
                        ks = nzk[m][wi]
                        for idx, ki in enumerate(ks):
                            ko, ksz = kch[ki]
                            nc.tensor.matmul(
                                ps[:Msz, :],
                                lhsT=rtt[m][:ksz, ki, Mo:Mo + Msz],
                                rhs=pct_t[:ksz, ki, :],
                                start=(idx == 0), stop=(idx == len(ks) - 1))
                    _pcopy(out=prc_sb[:Msz, wi, :], in_=ps[:Msz, :])

            st_out[m] = (stats, prc_sb, pct_t, _xview)

        # ---------- pass B: sigmoid/G/corr/C^T ----------
        def pass_b(spr, m, st_in, g_ps, gfirst, glast):
            c, sp = MEMBERS[m]
            stats, prc_sb, pct_t, _xview = st_in[m]

            hts = {}
            for si in range(2):
                h0 = hp.tile([128, 128], F16, tag=f"h0_{si}", name=f"h0_{si}")
                h1 = hp.tile([69, 128], F16, tag=f"h1_{si}", name=f"h1_{si}")
                nc.gpsimd.memset(h0, 0.0)
                nc.gpsimd.memset(h1, 0.0)
                bo = 16 * (4 * si + m)
                for wi, Msz in ((0, 128), (1, 68)):
                    isd, _ = stats[si, wi]
                    if m != 2:
                        src = prc_sb[:Msz, wi, 16 * si:16 * si + 16]
                    else:
                        src = pct_t[:Msz, wi, 16 * si:16 * si + 16]
                    dst = (h0 if wi == 0 else h1)[:Msz, bo:bo + Q]
                    nc.scalar.activation(out=dst, in_=src, func=AF.Sigmoid,
                                         scale=isd[:Msz])
                hts[si] = (h0, h1)

            for si in range(2):
                h0, h1 = hts[si]
                nc.tensor.matmul(g_ps, lhsT=h0, rhs=h0,
                                 start=(gfirst and si == 0), stop=False)
                nc.tensor.matmul(g_ps, lhsT=h1[:68, :], rhs=h1[:68, :],
                                 start=False, stop=(glast and si == 1))

            hss = {}
            for si in range(2):
                h0, h1 = hts[si]
                isd0, v0 = stats[si, 0]
                isd1, v1 = stats[si, 1]
                cr = ps3.tile([1, 128], F32, tag="ps")
                nc.tensor.matmul(cr[:1, :], lhsT=v0, rhs=h0,
                                 start=True, stop=False)
                nc.tensor.matmul(cr[:1, :], lhsT=v1[:68], rhs=h1[:68, :],
                                 start=False, stop=True)
                hs0 = hp.tile([128, 128], F16, tag=f"hs0_{si}",
                              name=f"hs0_{si}")
                hs1 = hp.tile([97, 128], F16, tag=f"hs1_{si}",
                              name=f"hs1_{si}")
                nc.gpsimd.memset(hs1[64:97, :], 0.0)
                nc.vector.tensor_scalar_mul(hs0, h0, isd0)
                nc.vector.tensor_scalar_mul(hs1[:68, :], h1[:68, :],
                                            isd1[:68])
                nc.vector.tensor_copy(out=hs1[96:97, :], in_=cr)
                hss[si] = (hs0, hs1)

            for no, nsz in _chunks(c, 1024):
                ct_ps = pacc.tile([128, 1024], F32, tag="pacc")
                for so, ssz in _chunks(nsz, 512):
                    idx = 0
                    for si in range(2):
                        hs0, hs1 = hss[si]
                        for wi in range(2):
                            lhsT = hs0 if wi == 0 else hs1
                            nc.tensor.matmul(
                                ct_ps[:, so:so + ssz], lhsT=lhsT,
                                rhs=_xview(si, wi, no + so, ssz),
                                start=(idx == 0), stop=(idx == 3))
                            idx += 1
                _pcopy(out=ct_all[spr][:, OFFS[m] + no:OFFS[m] + no + nsz],
                       in_=ct_ps[:, :nsz])

        # ---------------- Newton-Schulz generator ----------------
        def ns_gen(spr, g_ps):
            g = sup.tile([128, 128], F32, tag=f"g{spr}", name=f"g{spr}")
            nc.vector.tensor_copy(out=g, in_=g_ps)
            g_sb[spr] = g
            sq = newt.tile([128, 128], F32, tag="sq")
            nc.vector.tensor_mul(sq, g, g)
            rs = newt.tile([128, 1], F32, tag="rs")
            nc.vector.tensor_reduce(out=rs, in_=sq,
                                    axis=mybir.AxisListType.X, op=ALU.add)
            bps = ps3.tile([128, 64], F32, tag="ps")
            nc.tensor.matmul(bps[:128, 0:1], lhsT=p16_sb, rhs=rs,
                             start=True, stop=True)
            bf = newt.tile([128, 1], F32, tag="bf")
            nc.scalar.activation(out=bf, in_=bps[:128, 0:1], func=AF.Sqrt)
            al = newt.tile([128, 1], F32, tag="al")
            nc.vector.reciprocal(out=al, in_=bf)
            x_sb = newt.tile([128, 128], F32, tag="xns")
            nc.vector.tensor_scalar_mul(x_sb, ident_sb, al)
            yield
            for it in range(NEWTON_ITERS):
                yps = pacc.tile([128, 1024], F32, tag="pacc")
                nc.tensor.matmul(yps[:128, :128], lhsT=g, rhs=x_sb,
                                 start=True, stop=True)
                z_sb = newt.tile([128, 128], F32, tag="zns")
                nc.vector.scalar_tensor_tensor(
                    out=z_sb, in0=ident_sb, scalar=2.0,
                    in1=yps[:128, :128], op0=ALU.mult, op1=ALU.subtract)
                xps = pacc.tile([128, 1024], F32, tag="pacc")
                nc.tensor.matmul(xps[:128, :128], lhsT=x_sb, rhs=z_sb,
                                 start=True, stop=True)
                x_new = newt.tile([128, 128], F32, tag="xns")
                nc.scalar.copy(out=x_new, in_=xps[:128, :128])
                x_sb = x_new
                yield
            mps = pacc.tile([128, 1024], F32, tag="pacc")
            nc.tensor.matmul(mps[:128, :128], lhsT=x_sb, rhs=x_sb,
                             start=True, stop=True)
            m2t = sup.tile([128, 128], F16, tag=f"m2_{spr}", name=f"m2_{spr}")
            nc.vector.tensor_copy(out=m2t, in_=mps[:128, :128])
            m2_sb[spr] = m2t
            rps = ps3.tile([128, 64], F32, tag="ps")
            nc.tensor.matmul(rps[:128, 0:1], lhsT=x_sb, rhs=ones_sb,
                             start=True, stop=True)
            rt_ = sup.tile([128, 1], F32, tag=f"r_{spr}", name=f"r_{spr}")
            nc.vector.tensor_copy(out=rt_, in_=rps[:128, 0:1])
            r_sb[spr] = rt_
            yield
            for m in range(4):
                t = sup.tile([128, 2], F16, tag=f"r4_{spr}_{m}",
                             name=f"r4_{spr}_{m}")
                nc.vector.tensor_scalar_mul(t, mask8_sb[:, m, spr, :], rt_)
                r4_all[spr][m] = t
            yield

        # ---------------- phase-3 generator ----------------
        def ph3_gen(spr):
            rows = slice(2 * spr, 2 * spr + 2)
            for m, (c, sp) in enumerate(MEMBERS):
                otc = None
                if m == 3:
                    otc = outp.tile([2, 2048], F32, tag="otc", bufs=1)
                for no, nsz in _chunks(c, 512):
                    g0 = OFFS[m] + no
                    dfp = pacc.tile([128, 1024], F32, tag="pacc")
                    nc.tensor.matmul(
                        dfp[:, :nsz], lhsT=m2_sb[spr],
                        rhs=ct_all[spr][:, g0:g0 + nsz],
                        start=True, stop=True)
                    psb = outp.tile([128, 512], F16, tag="psb")
                    nc.vector.tensor_mul(psb[:, :nsz],
                                         ct_all[spr][:, g0:g0 + nsz],
                                         dfp[:, :nsz])
                    qps = ps3.tile([16, 512], F32, tag="ps")
                    nc.tensor.matmul(
                        qps[:2, :nsz], lhsT=oq8_sb[:, m, spr, :],
                        rhs=psb[:, :nsz], start=True, stop=True)
                    tps = ps3.tile([16, 512], F32, tag="ps")
                    nc.tensor.matmul(
                        tps[:2, :nsz], lhsT=r4_all[spr][m],
                        rhs=ct_all[spr][:, g0:g0 + nsz],
                        start=True, stop=True)
                    tsb = outp.tile([2, 512], F32, tag="tsb", bufs=1)
                    nc.vector.tensor_copy(out=tsb[:, :nsz], in_=tps[:2, :nsz])
                    ot = outp.tile([2, 512], F32, tag="ot", bufs=1)
                    nc.vector.tensor_mul(ot[:, :nsz], tsb[:, :nsz],
                                         tsb[:, :nsz])
                    nc.vector.scalar_tensor_tensor(
                        out=ot[:, :nsz],
                        in0=ot[:, :nsz], scalar=-1.0 / ((Q - 1) * Q),
                        in1=qps[:2, :nsz], op0=ALU.mult, op1=ALU.add)
                    if m == 3:
                        # un-permute (p k)-packed channels into otc:
                        # position e*1024 + j*128 + p  <->  channel 16p+2j+e
                        e = no // 1024
                        vw = otc.rearrange("r (p j e) -> r e j p",
                                           p=128, j=8, e=2)
                        j0 = (no % 1024) // 128
                        nc.vector.tensor_copy(
                            out=vw[:, e, j0:j0 + nsz // 128, :],
                            in_=ot[:, :nsz].rearrange("r (a b) -> r a b",
                                                      b=128))
                    else:
                        nc.sync.dma_start(out=outd[rows, g0:g0 + nsz],
                                          in_=ot[:, :nsz])
                    yield
                if m == 3:
                    nc.sync.dma_start(out=outd[rows, OFFS[3]:OFFS[3] + 2048],
                                      in_=otc)
                    yield

        # ================= driver =================
        gens = [None, None]
        ph3_0 = None
        for spr in range(2):
            xbf_all = {}
            for m in ORDER:
                c, sp = MEMBERS[m]
                uv = sp * sp
                for si in range(2):
                    s = 2 * spr + si
                    if m == 3:
                        raw = xbfp.tile([128, 784], F16, tag=f"xr3_{si}",
                                        name=f"xr3_{si}")
                        nc.gpsimd.dma_start(
                            out=raw,
                            in_=xin[3][s, :, :].rearrange(
                                "(p k) v -> p (k v)", p=128))
                        t = xbfp.tile([128, 16, 64], F16, tag=f"xbf3_{si}",
                                      name=f"xbf3_{si}")
                        nc.vector.memset(t[:, :, 49:64], 0.0)
                        nc.vector.tensor_copy(
                            out=t[:, :, 0:49],
                            in_=raw.rearrange("p (k v) -> p k v", k=16))
                        xbf_all[3, si] = t
                    else:
                        ccn = c // 128
                        nk = (uv + 127) // 128
                        uvp = nk * 128
                        t = xbfp.tile([128, ccn, uvp], F16,
                                      tag=f"xbf{m}_{si}", name=f"xbf{m}_{si}")
                        nc.gpsimd.dma_start(
                            out=t[:, :, :uv],
                            in_=xin[m][s, :, :].rearrange(
                                "(k p) v -> p k v", p=128))
                        if uv < uvp:
                            nc.gpsimd.memset(t[:, :, uv:uvp], 0.0)
                        xbf_all[m, si] = t
            g_ps = pg.tile([128, 128], F32, tag="pg")
            st = {}
            for mi, m in enumerate(ORDER):
                if spr == 1:
                    _drive(gens[0], 3)
                    _drive(ph3_0, 1)
                pass_a(spr, m, (xbf_all[m, 0], xbf_all[m, 1]), st)
            for mi, m in enumerate(ORDER):
                if spr == 1:
                    _drive(gens[0], 3)
                    if ph3_0 is None and r4_all[0][0] is not None:
                        ph3_0 = ph3_gen(0)
                    _drive(ph3_0, 2)
                pass_b(spr, m, st, g_ps, gfirst=(mi == 0), glast=(mi == 3))
            gens[spr] = ns_gen(spr, g_ps)
            _drive(gens[spr], 1)

        while _drive(gens[0], 1):
            pass
        if ph3_0 is None:
            ph3_0 = ph3_gen(0)
        more1 = True
        more0 = True
        while more0 or more1:
            more1 = _drive(gens[1], 1)
            more0 = _drive(ph3_0, 1)
        for _ in ph3_gen(1):
            pass

    nc.finalize()
    return nc


def _in_maps(xs, ws):
    rts, rtb, ident_np, identh_np, p16_np, mask8_np = _consts()
    waug = {}
    for m, (c, sp) in enumerate(MEMBERS):
        W = np.asarray(ws[m], np.float32)
        Wc = (W - W.sum(axis=1, keepdims=True) / c).T   # [c, Q]
        if m == 3:
            wa = Wc.reshape(128, 16, Q)                 # [p, k, q] = row 16p+k
        else:
            wa = Wc.reshape(c // 128, 128, Q).transpose(1, 0, 2)
        waug[m] = np.ascontiguousarray(wa.astype(NPF16))
    in_maps = []
    for i in range(NCORES):
        im = {"ident": ident_np, "identh": identh_np, "p16": p16_np,
              "mask8": mask8_np}
        for m, (c, sp) in enumerate(MEMBERS):
            im[f"x{m}"] = np.ascontiguousarray(
                xs[m][S * i:S * (i + 1)].reshape(S, c, sp * sp), np.float32)
            im[f"waug{m}"] = waug[m]
            if m in rtb:
                im[f"rt{m}"] = rtb[m]
        in_maps.append(im)
    return in_maps


_CACHE = {}


def kernel(x0, x1, x2, x3, W0, W1, W2, W3):
    if "nc" not in _CACHE:
        _CACHE["nc"] = _build_program()
    nc = _CACHE["nc"]
    xs = [np.asarray(x) for x in (x0, x1, x2, x3)]
    ws = [np.asarray(w) for w in (W0, W1, W2, W3)]
    in_maps = _in_maps(xs, ws)
    res = run_bass_kernel_spmd(nc, in_maps, list(range(NCORES)))
    return np.concatenate([r["out"] for r in res.results], axis=0)


# revision 21
# speedup vs baseline: 1.1782x; 1.1782x over previous
"""Trainium2 Bass kernel for nn_ELM_AE_FatSpectral_Ensemble.

Data-parallel over batch: 4 samples/core on 8 cores.  fp16 data path:
  x is cast fp32->fp16 during the input DMA (SWDGE), transposed to x^T via
  DMA XBAR transpose, resized on PE (XrT = Rk^T x^T).  zscore is folded:
    H   = sigmoid(isd .* resize(Wc @ x))     (Wc = W - rowmean(W), host-prep)
    C^T = (isd.*Hblk)^T XrT - corr x (-1 row)   (corr row at partition 96)
  Per super (2 samples): G = H H^T accumulated in PSUM as a 128x128
  block-diagonal supermatrix; Newton-Schulz inverse in fp32, interleaved at
  emission with the next super's processing.  Members are processed in two
  passes per super (A: loads/resize/stats+sqrt, B: sigmoid/G/C^T) so the
  scalar engine never alternates activation tables within a pass.
  m3 (c=2048, 7x7) is loaded packed ((p k) channel order) so its DMA uses
  1568-B lines instead of 98-B; its channel permutation is undone by an
  on-chip gather before the output DMA.
"""

import numpy as np

import concourse.bacc as bacc
import concourse.tile as tile
from concourse import mybir
from concourse.bass_utils import run_bass_kernel_spmd

F32 = mybir.dt.float32
F32R = mybir.dt.float32r
F16 = mybir.dt.float16
AF = mybir.ActivationFunctionType
ALU = mybir.AluOpType
NPF16 = np.float16

S = 4
NCORES = 8
SP = 14
WH = SP * SP
Q = 16
MEMBERS = [(256, 56), (512, 28), (1024, 14), (2048, 7)]
OFFS = [0, 256, 768, 1792]
DTOT = 3840
ORDER = [3, 2, 1, 0]
NEWTON_ITERS = 10


def _weight_mat(n_in, n_out):
    scale = n_out / n_in
    kernel_scale = max(1.0, 1.0 / scale)
    sample_f = (np.arange(n_out) + 0.5) / scale - 0.5
    x = np.abs(sample_f[:, None] - np.arange(n_in)[None, :]) / kernel_scale
    w = np.maximum(0.0, 1.0 - x)
    total = w.sum(axis=1, keepdims=True)
    return (w / np.where(total > 0, total, 1)).astype(np.float32)


def _chunks(n, sz=128):
    return [(i, min(sz, n - i)) for i in range(0, n, sz)]


def _consts():
    rts = {}
    for m, (c, sp) in enumerate(MEMBERS):
        if sp == SP:
            continue
        R = _weight_mat(sp, SP)
        rts[m] = np.kron(R, R).T.astype(np.float32)   # [uv, 196]
    rtb = {}
    for m in (0, 1):
        rt = rts[m]
        pad = (-rt.shape[0]) % 128
        rtp = np.concatenate([rt, np.zeros((pad, WH), np.float32)], 0)
        rtb[m] = np.ascontiguousarray(
            rtp.reshape(-1, 128, WH).transpose(1, 0, 2).astype(NPF16))
    # m3: rows 0:49 and 64:113 both hold rt3 (for base-64 parity matmuls)
    rt3 = np.zeros((128, 1, WH), np.float32)
    rt3[0:49, 0] = rts[3]
    rt3[64:113, 0] = rts[3]
    rtb[3] = np.ascontiguousarray(rt3.astype(NPF16))
    ident = np.eye(128, dtype=np.float32)
    identh = np.eye(128, dtype=NPF16)
    p16 = np.kron(np.eye(8, dtype=np.float32), np.ones((16, 16), np.float32))
    mask8 = np.zeros((4, 2, 128, 2), np.float32)
    for m in range(4):
        for spr in range(2):
            for si in range(2):
                b = 4 * si + m
                mask8[m, spr, 16 * b:16 * b + 16, si] = 1.0
    mask8 = np.ascontiguousarray(mask8.transpose(2, 0, 1, 3))  # [128,4,2,2]
    return rts, rtb, ident, identh, p16, mask8


def _drive(gen, n):
    if gen is None:
        return False
    for _ in range(n):
        try:
            next(gen)
        except StopIteration:
            return False
    return True


def _build_program():
    rts, rtb, ident_np, identh_np, p16_np, mask8_np = _consts()

    nc = bacc.Bacc()
    xin, waugd, rtd = {}, {}, {}
    for m, (c, sp) in enumerate(MEMBERS):
        uv = sp * sp
        xin[m] = nc.dram_tensor(f"x{m}", [S, c, uv], F32R, kind="ExternalInput")
        nw = 16 if m == 3 else c // 128
        waugd[m] = nc.dram_tensor(f"waug{m}", [128, nw, Q], F16,
                                  kind="ExternalInput")
        if m in rtb:
            rtd[m] = nc.dram_tensor(f"rt{m}", list(rtb[m].shape), F16,
                                    kind="ExternalInput")
    identd = nc.dram_tensor("ident", [128, 128], F32, kind="ExternalInput")
    identhd = nc.dram_tensor("identh", [128, 128], F16, kind="ExternalInput")
    p16d = nc.dram_tensor("p16", [128, 128], F32, kind="ExternalInput")
    mask8d = nc.dram_tensor("mask8", [128, 4, 2, 2], F32, kind="ExternalInput")
    outd = nc.dram_tensor("out", [S, DTOT], F32, kind="ExternalOutput")

    nzk = {}
    for m in (0, 1):
        uv = MEMBERS[m][1] ** 2
        nzk[m] = {}
        for Mi, (Mo, Msz) in enumerate([(0, 128), (128, 68)]):
            nzk[m][Mi] = [ki for ki, (ko, ksz) in enumerate(_chunks(uv))
                          if np.any(rts[m][ko:ko + ksz, Mo:Mo + Msz] != 0)]

    from contextlib import ExitStack
    _ceng = [0]
    _teng = [0]

    def _pcopy(out, in_):
        _ceng[0] ^= 1
        if _ceng[0]:
            nc.scalar.copy(out=out, in_=in_)
        else:
            nc.vector.tensor_copy(out=out, in_=in_)

    def _tdma(out, in_):
        nc.sync.dma_start(out=out, in_=in_, transpose=True)

    with tile.TileContext(nc) as tc, ExitStack() as _es:
        _p = lambda **kw: _es.enter_context(tc.tile_pool(**kw))
        consts = _p(name="consts", bufs=1)
        xbfp = _p(name="xbfp", bufs=1)
        xtp = _p(name="xtp", bufs=1)
        xrtp = _p(name="xrtp", bufs=1)
        pstp = _p(name="pstp", bufs=1)
        pctp = _p(name="pctp", bufs=1)
        prcp = _p(name="prcp", bufs=1)
        hp = _p(name="hp", bufs=2)
        smalls = _p(name="smalls", bufs=3)
        sup = _p(name="sup", bufs=1)
        newt = _p(name="newt", bufs=2)
        outp = _p(name="outp", bufs=2)
        pacc = _p(name="pacc", bufs=2, space="PSUM")   # [128,1024]: 4 banks
        ps3 = _p(name="ps3", bufs=3, space="PSUM")     # 3 banks shared tag
        pg = _p(name="pg", bufs=1, space="PSUM")       # 1 bank

        # ---------------- constants ----------------
        ident_sb = consts.tile([128, 128], F32, tag="ident")
        nc.sync.dma_start(out=ident_sb, in_=identd[:, :])
        identh_sb = consts.tile([128, 128], F16, tag="identh")
        nc.sync.dma_start(out=identh_sb, in_=identhd[:, :])
        p16_sb = consts.tile([128, 128], F32, tag="p16")
        nc.sync.dma_start(out=p16_sb, in_=p16d[:, :])
        mask8_sb = consts.tile([128, 4, 2, 2], F32, tag="mask8")
        nc.sync.dma_start(out=mask8_sb, in_=mask8d[:, :, :, :])
        oq8_sb = consts.tile([128, 4, 2, 2], F16, tag="oq8")
        nc.vector.tensor_scalar_mul(
            oq8_sb.rearrange("p a b c -> p (a b c)"),
            mask8_sb.rearrange("p a b c -> p (a b c)"), 1.0 / (Q - 1))
        ones_sb = consts.tile([128, 1], F32, tag="ones")
        nc.vector.memset(ones_sb, 1.0)

        rtt, waug_sb = {}, {}
        for m, (c, sp) in enumerate(MEMBERS):
            nw = 16 if m == 3 else c // 128
            waug_sb[m] = consts.tile([128, nw, Q], F16, tag=f"waug{m}",
                                     name=f"waug{m}")
            nc.sync.dma_start(out=waug_sb[m], in_=waugd[m][:, :, :])
            if m in rtb:
                nk = rtb[m].shape[1]
                rtt[m] = consts.tile([128, nk, WH], F16, tag=f"rt{m}",
                                     name=f"rt{m}")
                nc.sync.dma_start(out=rtt[m], in_=rtd[m][:, :, :])

        ct_all = []
        for spr in range(2):
            t = sup.tile([128, DTOT], F16, tag=f"ct{spr}", name=f"ct{spr}")
            ct_all.append(t)
        g_sb = [None, None]
        m2_sb = [None, None]
        r_sb = [None, None]
        r4_all = [[None] * 4, [None] * 4]

        # ---------- pass A: load/transpose/P/resize/stats ----------
        def pass_a(spr, m, xbf, st_out):
            c, sp = MEMBERS[m]
            uv = sp * sp
            ccn = 16 if m == 3 else c // 128
            if m != 3:
                kch = _chunks(uv)
                nk = len(kch)

            # x^T via DMA XBAR transpose
            xts = {}
            for si in range(2):
                if m == 3:
                    xt = xtp.tile([128, 8, 128], F16, tag=f"xt3_{si}",
                                  name=f"xt3_{si}")
                    _tdma(out=xt, in_=xbf[si].rearrange("p a b -> p (a b)"))
                    xb = xtp.tile([49, 8, 128], F16, tag=f"xt3b_{si}",
                                  name=f"xt3b_{si}")
                    nc.vector.tensor_copy(out=xb, in_=xt[64:113, :, :])
                    xts[si, "b"] = xb
                else:
                    xt = xtp.tile([128, ccn, nk, 128], F16, tag=f"xt{m}_{si}",
                                  name=f"xt{m}_{si}")
                    _tdma(out=xt, in_=xbf[si].rearrange("p a b -> p (a b)"))
                xts[si] = xt

            # P = Wc @ x
            uvp = 128 if m == 3 else nk * 128
            pst = {}
            for si in range(2):
                pst[si] = pstp.tile([16, uvp], F16, tag=f"pst{m}_{si}",
                                    name=f"pst{m}_{si}")
                if uv < uvp:
                    nc.gpsimd.memset(pst[si][:, uv:uvp], 0.0)
                for no, nsz in _chunks(uv, 512):
                    pp_ps = ps3.tile([16, 512], F32, tag="ps")
                    for cc in range(ccn):
                        nc.tensor.matmul(
                            pp_ps[:, :nsz], lhsT=waug_sb[m][:, cc, :],
                            rhs=xbf[si][:, cc, no:no + nsz],
                            start=(cc == 0), stop=(cc == ccn - 1))
                    _pcopy(out=pst[si][:, no:no + nsz],
                           in_=pp_ps[:Q, :nsz])

            # Pc^T via DMA XBAR transpose (tiny)
            pnk = 1 if m == 3 else nk
            pct_t = pctp.tile([128, pnk, 32], F16, tag=f"pct{m}",
                              name=f"pct{m}")
            for si in range(2):
                _tdma(out=pct_t[:, :, 16 * si:16 * si + 16], in_=pst[si])

            # resize -> XrT (m2: direct view of x^T)
            xrt = {}
            if m == 2:
                for si in range(2):
                    nc.gpsimd.memset(xts[si][96:97, :, 1, :], -1.0)
            else:
                for si in range(2):
                    x0t = xrtp.tile([128, c], F16, tag=f"x0t{m}_{si}",
                                    name=f"x0t{m}_{si}")
                    x1t = xrtp.tile([97, c], F16, tag=f"x1t{m}_{si}",
                                    name=f"x1t{m}_{si}")
                    nc.gpsimd.memset(x1t[64:97, :], 0.0)
                    nc.gpsimd.memset(x1t[96:97, :], -1.0)
                    for wi, (Mo, Msz, dst) in enumerate(
                            ((0, 128, x0t), (128, 68, x1t))):
                        if m == 3:
                            for e in range(2):
                                src3 = (xts[si] if e == 0
                                        else xts[si, "b"])
                                pr = pacc.tile([128, 1024], F32, tag="pacc")
                                for h in range(2):
                                    nc.tensor.matmul(
                                        pr[:Msz, 512 * h:512 * (h + 1)],
                                        lhsT=rtt[3][0:49, 0, Mo:Mo + Msz],
                                        rhs=src3[0:49, 4 * h:4 * h + 4, :],
                                        start=True, stop=True)
                                _pcopy(out=dst[:Msz,
                                               1024 * e:1024 * (e + 1)],
                                       in_=pr[:Msz, :])
                        else:
                            for no, nsz in _chunks(c, 1024):
                                pr = pacc.tile([128, 1024], F32, tag="pacc")
                                ks = nzk[m][wi]
                                for so, ssz in _chunks(nsz, 512):
                                    cc0 = (no + so) // 128
                                    ncc = (ssz + 127) // 128
                                    for idx, ki in enumerate(ks):
                                        ko, ksz = kch[ki]
                                        nc.tensor.matmul(
                                            pr[:Msz, so:so + ssz],
                                            lhsT=rtt[m][:ksz, ki,
                                                        Mo:Mo + Msz],
                                            rhs=xts[si][:ksz, cc0:cc0 + ncc,
                                                        ki, :],
                                            start=(idx == 0),
                                            stop=(idx == len(ks) - 1))
                                _pcopy(out=dst[:Msz, no:no + nsz],
                                       in_=pr[:Msz, :nsz])
                    xrt[si, 0] = x0t
                    xrt[si, 1] = x1t

            def _xview(si, wi, no, nsz):
                hi = 128 if wi == 0 else 97
                if m != 2:
                    return xrt[si, wi][:hi, no:no + nsz]
                cc0 = no // 128
                ncc = (nsz + 127) // 128
                return xts[si][:hi, cc0:cc0 + ncc, wi, :]

            # stats: mu, isd, v = isd*mu
            stats = {}
            for si in range(2):
                for wi, psz in ((0, 128), (1, 68)):
                    if m != 2:
                        srcs = [xrt[si, wi][:psz, go:go + gln]
                                for go, gln in _chunks(c, 512)]
                    else:
                        srcs = [xts[si][:psz, cc, wi, :]
                                for cc in range(c // 128)]
                    st = smalls.tile([128, len(srcs), 6], F32, tag="bnst")
                    for gi, src in enumerate(srcs):
                        nc.vector.bn_stats(out=st[:psz, gi, :], in_=src)
                    mv = smalls.tile([128, 2], F32, tag="mv")
                    nc.vector.bn_aggr(out=mv[:psz], in_=st[:psz])
                    sd = smalls.tile([128, 1], F32, tag="sd")
                    nc.scalar.activation(out=sd[:psz], in_=mv[:psz, 1:2],
                                         func=AF.Sqrt, scale=c / (c - 1.0))
                    isd = smalls.tile([128, 1], F32, tag="isd", bufs=18)
                    nc.vector.reciprocal(out=isd[:psz], in_=sd[:psz])
                    v = smalls.tile([128, 1], F16, tag="vv", bufs=18)
                    nc.vector.tensor_mul(v[:psz], mv[:psz, 0:1], isd[:psz])
                    stats[si, wi] = (isd, v)

            # PRc = Rk^T Pc^T -> SBUF fp32 (m2: pct_t used directly)
            prc_sb = None
            if m != 2:
                prc_sb = prcp.tile([128, 2, 32], F32, tag=f"prc{m}",
                                   name=f"prc{m}")
                for wi, (Mo, Msz) in enumerate(((0, 128), (128, 68))):
                    ps = ps3.tile([128, 32], F32, tag="ps")
                    if m == 3:
                        nc.tensor.matmul(
                            ps[:Msz, :], lhsT=rtt[3][0:49, 0, Mo:Mo + Msz],
                            rhs=pct_t[0:49, 0, :], start=True, stop=True)
                    else:
                        ks = nzk[m][wi]
                        for idx, ki in enumerate(ks):
                            ko, ksz = kch[ki]
                            nc.tensor.matmul(
                                ps[:Msz, :],
                                lhsT=rtt[m][:ksz, ki, Mo:Mo + Msz],
                                rhs=pct_t[:ksz, ki, :],
                                start=(idx == 0), stop=(idx == len(ks) - 1))
                    _pcopy(out=prc_sb[:Msz, wi, :], in_=ps[:Msz, :])

            st_out[m] = (stats, prc_sb, pct_t, _xview)

        # ---------- pass B: sigmoid/G/corr/C^T ----------
        def pass_b(spr, m, st_in, g_ps, gfirst, glast):
            c, sp = MEMBERS[m]
            stats, prc_sb, pct_t, _xview = st_in[m]

            hts = {}
            for si in range(2):
                h0 = hp.tile([128, 128], F16, tag=f"h0_{si}", name=f"h0_{si}")
                h1 = hp.tile([69, 128], F16, tag=f"h1_{si}", name=f"h1_{si}")
                nc.gpsimd.memset(h0, 0.0)
                nc.gpsimd.memset(h1, 0.0)
                bo = 16 * (4 * si + m)
                for wi, Msz in ((0, 128), (1, 68)):
                    isd, _ = stats[si, wi]
                    if m != 2:
                        src = prc_sb[:Msz, wi, 16 * si:16 * si + 16]
                    else:
                        src = pct_t[:Msz, wi, 16 * si:16 * si + 16]
                    dst = (h0 if wi == 0 else h1)[:Msz, bo:bo + Q]
                    nc.scalar.activation(out=dst, in_=src, func=AF.Sigmoid,
                                         scale=isd[:Msz])
                hts[si] = (h0, h1)

            for si in range(2):
                h0, h1 = hts[si]
                nc.tensor.matmul(g_ps, lhsT=h0, rhs=h0,
                                 start=(gfirst and si == 0), stop=False)
                nc.tensor.matmul(g_ps, lhsT=h1[:68, :], rhs=h1[:68, :],
                                 start=False, stop=(glast and si == 1))

            hss = {}
            for si in range(2):
                h0, h1 = hts[si]
                isd0, v0 = stats[si, 0]
                isd1, v1 = stats[si, 1]
                cr = ps3.tile([1, 128], F32, tag="ps")
                nc.tensor.matmul(cr[:1, :], lhsT=v0, rhs=h0,
                                 start=True, stop=False)
                nc.tensor.matmul(cr[:1, :], lhsT=v1[:68], rhs=h1[:68, :],
                                 start=False, stop=True)
                hs0 = hp.tile([128, 128], F16, tag=f"hs0_{si}",
                              name=f"hs0_{si}")
                hs1 = hp.tile([97, 128], F16, tag=f"hs1_{si}",
                              name=f"hs1_{si}")
                nc.gpsimd.memset(hs1[64:97, :], 0.0)
                nc.vector.tensor_scalar_mul(hs0, h0, isd0)
                nc.vector.tensor_scalar_mul(hs1[:68, :], h1[:68, :],
                                            isd1[:68])
                nc.vector.tensor_copy(out=hs1[96:97, :], in_=cr)
                hss[si] = (hs0, hs1)

            for no, nsz in _chunks(c, 1024):
                ct_ps = pacc.tile([128, 1024], F32, tag="pacc")
                for so, ssz in _chunks(nsz, 512):
                    idx = 0
                    for si in range(2):
                        hs0, hs1 = hss[si]
                        for wi in range(2):
                            lhsT = hs0 if wi == 0 else hs1
                            nc.tensor.matmul(
                                ct_ps[:, so:so + ssz], lhsT=lhsT,
                                rhs=_xview(si, wi, no + so, ssz),
                                start=(idx == 0), stop=(idx == 3))
                            idx += 1
                _pcopy(out=ct_all[spr][:, OFFS[m] + no:OFFS[m] + no + nsz],
                       in_=ct_ps[:, :nsz])

        # ---------------- Newton-Schulz generator ----------------
        def ns_gen(spr, g_ps):
            g = sup.tile([128, 128], F32, tag=f"g{spr}", name=f"g{spr}")
            nc.vector.tensor_copy(out=g, in_=g_ps)
            g_sb[spr] = g
            sq = newt.tile([128, 128], F32, tag="sq")
            nc.vector.tensor_mul(sq, g, g)
            rs = newt.tile([128, 1], F32, tag="rs")
            nc.vector.tensor_reduce(out=rs, in_=sq,
                                    axis=mybir.AxisListType.X, op=ALU.add)
            bps = ps3.tile([128, 64], F32, tag="ps")
            nc.tensor.matmul(bps[:128, 0:1], lhsT=p16_sb, rhs=rs,
                             start=True, stop=True)
            bf = newt.tile([128, 1], F32, tag="bf")
            nc.scalar.activation(out=bf, in_=bps[:128, 0:1], func=AF.Sqrt)
            al = newt.tile([128, 1], F32, tag="al")
            nc.vector.reciprocal(out=al, in_=bf)
            x_sb = newt.tile([128, 128], F32, tag="xns")
            nc.vector.tensor_scalar_mul(x_sb, ident_sb, al)
            yield
            for it in range(NEWTON_ITERS):
                yps = pacc.tile([128, 1024], F32, tag="pacc")
                nc.tensor.matmul(yps[:128, :128], lhsT=g, rhs=x_sb,
                                 start=True, stop=True)
                z_sb = newt.tile([128, 128], F32, tag="zns")
                nc.vector.scalar_tensor_tensor(
                    out=z_sb, in0=ident_sb, scalar=2.0,
                    in1=yps[:128, :128], op0=ALU.mult, op1=ALU.subtract)
                xps = pacc.tile([128, 1024], F32, tag="pacc")
                nc.tensor.matmul(xps[:128, :128], lhsT=x_sb, rhs=z_sb,
                                 start=True, stop=True)
                x_new = newt.tile([128, 128], F32, tag="xns")
                nc.scalar.copy(out=x_new, in_=xps[:128, :128])
                x_sb = x_new
                yield
            mps = pacc.tile([128, 1024], F32, tag="pacc")
            nc.tensor.matmul(mps[:128, :128], lhsT=x_sb, rhs=x_sb,
                             start=True, stop=True)
            m2t = sup.tile([128, 128], F16, tag=f"m2_{spr}", name=f"m2_{spr}")
            nc.vector.tensor_copy(out=m2t, in_=mps[:128, :128])
            m2_sb[spr] = m2t
            rps = ps3.tile([128, 64], F32, tag="ps")
            nc.tensor.matmul(rps[:128, 0:1], lhsT=x_sb, rhs=ones_sb,
                             start=True, stop=True)
            rt_ = sup.tile([128, 1], F32, tag=f"r_{spr}", name=f"r_{spr}")
            nc.vector.tensor_copy(out=rt_, in_=rps[:128, 0:1])
            r_sb[spr] = rt_
            yield
            for m in range(4):
                t = sup.tile([128, 2], F16, tag=f"r4_{spr}_{m}",
                             name=f"r4_{spr}_{m}")
                nc.vector.tensor_scalar_mul(t, mask8_sb[:, m, spr, :], rt_)
                r4_all[spr][m] = t
            yield

        # ---------------- phase-3 generator ----------------
        def ph3_gen(spr):
            rows = slice(2 * spr, 2 * spr + 2)
            for m, (c, sp) in enumerate(MEMBERS):
                otc = None
                if m == 3:
                    otc = outp.tile([2, 2048], F32, tag="otc", bufs=1)
                for no, nsz in _chunks(c, 512):
                    g0 = OFFS[m] + no
                    dfp = pacc.tile([128, 1024], F32, tag="pacc")
                    nc.tensor.matmul(
                        dfp[:, :nsz], lhsT=m2_sb[spr],
                        rhs=ct_all[spr][:, g0:g0 + nsz],
                        start=True, stop=True)
                    psb = outp.tile([128, 512], F16, tag="psb")
                    nc.vector.tensor_mul(psb[:, :nsz],
                                         ct_all[spr][:, g0:g0 + nsz],
                                         dfp[:, :nsz])
                    qps = ps3.tile([16, 512], F32, tag="ps")
                    nc.tensor.matmul(
                        qps[:2, :nsz], lhsT=oq8_sb[:, m, spr, :],
                        rhs=psb[:, :nsz], start=True, stop=True)
                    tps = ps3.tile([16, 512], F32, tag="ps")
                    nc.tensor.matmul(
                        tps[:2, :nsz], lhsT=r4_all[spr][m],
                        rhs=ct_all[spr][:, g0:g0 + nsz],
                        start=True, stop=True)
                    tsb = outp.tile([2, 512], F32, tag="tsb", bufs=1)
                    nc.vector.tensor_copy(out=tsb[:, :nsz], in_=tps[:2, :nsz])
                    ot = outp.tile([2, 512], F32, tag="ot", bufs=1)
                    nc.vector.tensor_mul(ot[:, :nsz], tsb[:, :nsz],
                                         tsb[:, :nsz])
                    nc.vector.scalar_tensor_tensor(
                        out=ot[:, :nsz],
                        in0=ot[:, :nsz], scalar=-1.0 / ((Q - 1) * Q),
                        in1=qps[:2, :nsz], op0=ALU.mult, op1=ALU.add)
                    if m == 3:
                        # un-permute (p k)-packed channels into otc:
                        # position e*1024 + j*128 + p  <->  channel 16p+2j+e
                        e = no // 1024
                        vw = otc.rearrange("r (p j e) -> r e j p",
                                           p=128, j=8, e=2)
                        j0 = (no % 1024) // 128
                        nc.vector.tensor_copy(
                            out=vw[:, e, j0:j0 + nsz // 128, :],
                            in_=ot[:, :nsz].rearrange("r (a b) -> r a b",
                                                      b=128))
                    else:
                        nc.sync.dma_start(out=outd[rows, g0:g0 + nsz],
                                          in_=ot[:, :nsz])
                    yield
                if m == 3:
                    nc.sync.dma_start(out=outd[rows, OFFS[3]:OFFS[3] + 2048],
                                      in_=otc)
                    yield

        # ================= driver =================
        gens = [None, None]
        ph3_0 = None
        for spr in range(2):
            xbf_all = {}
            for m in ORDER:
                c, sp = MEMBERS[m]
                uv = sp * sp
                for si in range(2):
                    s = 2 * spr + si
                    if m == 3:
                        raw = xbfp.tile([128, 784], F16, tag=f"xr3_{si}",
                                        name=f"xr3_{si}")
                        nc.gpsimd.dma_start(
                            out=raw,
                            in_=xin[3][s, :, :].rearrange(
                                "(p k) v -> p (k v)", p=128))
                        t = xbfp.tile([128, 16, 64], F16, tag=f"xbf3_{si}",
                                      name=f"xbf3_{si}")
                        nc.gpsimd.memset(t[:, :, 49:64], 0.0)
                        nc.vector.tensor_copy(
                            out=t[:, :, 0:49],
                            in_=raw.rearrange("p (k v) -> p k v", k=16))
                        xbf_all[3, si] = t
                    else:
                        ccn = c // 128
                        nk = (uv + 127) // 128
                        uvp = nk * 128
                        t = xbfp.tile([128, ccn, uvp], F16,
                                      tag=f"xbf{m}_{si}", name=f"xbf{m}_{si}")
                        nc.gpsimd.dma_start(
                            out=t[:, :, :uv],
                            in_=xin[m][s, :, :].rearrange(
                                "(k p) v -> p k v", p=128))
                        if uv < uvp:
                            nc.gpsimd.memset(t[:, :, uv:uvp], 0.0)
                        xbf_all[m, si] = t
            g_ps = pg.tile([128, 128], F32, tag="pg")
            st = {}
            for mi, m in enumerate(ORDER):
                if spr == 1:
                    _drive(gens[0], 3)
                    _drive(ph3_0, 1)
                pass_a(spr, m, (xbf_all[m, 0], xbf_all[m, 1]), st)
            for mi, m in enumerate(ORDER):
                if spr == 1:
                    _drive(gens[0], 3)
                    if ph3_0 is None and r4_all[0][0] is not None:
                        ph3_0 = ph3_gen(0)
                    _drive(ph3_0, 2)
                pass_b(spr, m, st, g_ps, gfirst=(mi == 0), glast=(mi == 3))
            gens[spr] = ns_gen(spr, g_ps)
            _drive(gens[spr], 1)

        while _drive(gens[0], 1):
            pass
        if ph3_0 is None:
            ph3_0 = ph3_gen(0)
        more1 = True
        more0 = True
        while more0 or more1:
            more1 = _drive(gens[1], 1)
            more0 = _drive(ph3_0, 1)
        for _ in ph3_gen(1):
            pass

    nc.finalize()
    return nc


def _in_maps(xs, ws):
    rts, rtb, ident_np, identh_np, p16_np, mask8_np = _consts()
    waug = {}
    for m, (c, sp) in enumerate(MEMBERS):
        W = np.asarray(ws[m], np.float32)
        Wc = (W - W.sum(axis=1, keepdims=True) / c).T   # [c, Q]
        if m == 3:
            wa = Wc.reshape(128, 16, Q)                 # [p, k, q] = row 16p+k
        else:
            wa = Wc.reshape(c // 128, 128, Q).transpose(1, 0, 2)
        waug[m] = np.ascontiguousarray(wa.astype(NPF16))
    in_maps = []
    for i in range(NCORES):
        im = {"ident": ident_np, "identh": identh_np, "p16": p16_np,
              "mask8": mask8_np}
        for m, (c, sp) in enumerate(MEMBERS):
            im[f"x{m}"] = np.ascontiguousarray(
                xs[m][S * i:S * (i + 1)].reshape(S, c, sp * sp), np.float32)
            im[f"waug{m}"] = waug[m]
            if m in rtb:
                im[f"rt{m}"] = rtb[m]
        in_maps.append(im)
    return in_maps


_CACHE = {}


def kernel(x0, x1, x2, x3, W0, W1, W2, W3):
    if "nc" not in _CACHE:
        _CACHE["nc"] = _build_program()
    nc = _CACHE["nc"]
    xs = [np.asarray(x) for x in (x0, x1, x2, x3)]
    ws = [np.asarray(w) for w in (W0, W1, W2, W3)]
    in_maps = _in_maps(xs, ws)
    res = run_bass_kernel_spmd(nc, in_maps, list(range(NCORES)))
    return np.concatenate([r["out"] for r in res.results], axis=0)
